# revision 16
# baseline (speedup 1.0000x reference)
"""Trainium2 Bass kernel for CSDrafting speculative-decoding verification.

Problem: B=1, L=520, V=32128, review_index r=512, K=8 draft tokens, 8 cores.

Work decomposition (row-sharding instead of the vocab-sharding hint — no
collectives needed):
  - prob_res rows 0..511 are a verbatim copy of probs rows 0..511: sharded
    row-wise, 64 rows per core, as 8 independent 1 MB HBM->SBUF->HBM DMA
    pairs (they overlap; only the 2-deep chain latency shows).
  - The "special" math (softmax+argmax of logits rows 511..519, accept flags,
    n_match, masked softmax rows 512..519, new ids) runs redundantly on every
    core; the host takes core 0's copy.

This axon terminal charges a large fixed latency (~30-60us) per instruction
(PE matmul ~200us, broadcast-operand reads +45us) and cannot load custom
DVE-table ops, so the design minimizes the dependency-chain depth using only
standard BIR ops:
  - logits rows live as (9 rows x 8 chunks = 72 partitions, 4016 free) so
    every per-row comparison uses per-partition scalars (no broadcasts),
  - exp + per-chunk sumexp fuse into one activation with accum_out,
  - (mask==max)*index + its reduction fuse into one scalar_tensor_tensor
    with accum_out; same for the cross-chunk winner select,
  - cross-partition steps use two tiny SBUF->SBUF DMA partition-collapses,
  - n_match = min_j(j + K*accept_j) replaces the cumprod scan,
  - the per-row output scale expands to 64 partitions via one free-dim
    broadcast op + one partition-spread DMA, and the final masked-softmax
    multiply runs on the Scalar engine (activation Copy with per-partition
    scale), off the Vector engine's critical chain.
  - the 16 per-draft scalars the reference gathers (probs/logits at draft
    ids) are host-gathered into the constants row (pure input marshaling;
    all O(V) compute stays on device).
"""

import sys

sys.path.insert(0, "/opt/trn_rl_repo")

import numpy as np

import concourse.bacc as bacc
import concourse.mybir as mybir
from concourse.tile import TileContext

F32 = mybir.dt.float32
AX = mybir.AxisListType
OP = mybir.AluOpType
ACT = mybir.ActivationFunctionType

V = 32128
L = 520
R = 512
K = 8
NR = K + 1  # 9 rows under review
CH = 8  # vocab chunks per row
F = V // CH  # 4016 free elems per chunk
NP = NR * CH  # 72 partitions of logits data
NCORES = 8
ROWS_PER_CORE = R // NCORES  # 64

# offsets inside the constants row (row 72 of sp2)
O_DSC = 0  # [0:8]    draft ids / 4096
O_LGD = 8  # [8:16]   exp(lgd)*leniency/pp  (lenient accept threshold)
O_POS = 24  # [24:33]  arange(9)
O_DPAD = 33  # [33:42]  draft ids raw, last = 0
O_POS64 = 64  # [64:128] q//8 for q in 0..63
IDSCALE = 4096.0


def build_nc(niter=1, internal_bulk=False, parts=("copy", "special"), leniency=2, stage=5):
    nc = bacc.Bacc("TRN2", target_bir_lowering=False, debug=False, num_devices=NCORES)

    if internal_bulk:
        psrc = nc.dram_tensor("ipsrc", [ROWS_PER_CORE, V], F32)
        pdst = nc.dram_tensor("ipdst", [ROWS_PER_CORE, V], F32)
        spec = nc.dram_tensor("ispec", [K, V], F32)
    else:
        psrc = nc.dram_tensor("psrc", [ROWS_PER_CORE, V], F32, kind="ExternalInput")
        pdst = nc.dram_tensor("pdst", [ROWS_PER_CORE, V], F32, kind="ExternalOutput")
        spec = nc.dram_tensor("spec", [K, V], F32, kind="ExternalOutput")
    sp2 = nc.dram_tensor("sp2", [2 * NP + 1, F], F32, kind="ExternalInput")
    sout = nc.dram_tensor("sout", [16], F32, kind="ExternalOutput")

    lgate = 1.0 if leniency > 1 else 0.0
    lenf = float(leniency)

    with TileContext(nc) as tc:
        with tc.tile_pool(name="pool", bufs=1) as pool:
            for _ in range(niter):
                if "special" in parts:
                    # logits chunks, constants row (own partition-0 tile,
                    # engine ops must start at partition 0/32/64/96), index rows
                    SPT = pool.tile([NP, F], F32)
                    nc.sync.dma_start(out=SPT[:, :], in_=sp2[0:NP, :])
                    CONT = pool.tile([1, 128], F32)
                    nc.sync.dma_start(out=CONT[0:1, :], in_=sp2[NP : NP + 1, 0:128])
                    IOT = pool.tile([NP, F], F32)
                    nc.sync.dma_start(out=IOT[:, :], in_=sp2[NP + 1 : 2 * NP + 1, :])
                    CON = CONT[0:1, :]

                    COLT = pool.tile([NP, 3], F32)
                    # exp + per-chunk sumexp in one op
                    EXP = pool.tile([NP, F], F32)
                    nc.scalar.activation(
                        EXP[:, :], SPT[:, :], ACT.Exp, accum_out=COLT[:, 2:3]
                    )
                    if stage >= 2:
                        # per-chunk max, then fused (x==max)*index with accum
                        nc.vector.tensor_reduce(
                            COLT[0:NP, 0:1], SPT[0:NP, :], axis=AX.X, op=OP.max
                        )
                        SCR = pool.tile([NP, F], F32)
                        nc.vector.scalar_tensor_tensor(
                            SCR[:, :],
                            SPT[0:NP, :],
                            COLT[0:NP, 0:1],
                            IOT[:, :],
                            op0=OP.is_equal,
                            op1=OP.mult,
                            accum_out=COLT[0:NP, 1:2],
                        )
                    if stage >= 3:
                        # collapse (72,3) -> (9 rows, 8 chunks, 3)
                        COLR = pool.tile([NR, CH, 3], F32)
                        nc.sync.dma_start(out=COLR[:, :, :], in_=COLT[0:NP, :])
                        G9 = pool.tile([NR, 1], F32)
                        nc.vector.tensor_reduce(
                            G9[:, :], COLR[:, :, 0], axis=AX.X, op=OP.max
                        )
                        T92 = pool.tile([NR, 2], F32)
                        W9 = pool.tile([NR, CH], F32)
                        nc.vector.scalar_tensor_tensor(
                            W9[:, :],
                            COLR[:, :, 0],
                            G9[:, 0:1],
                            COLR[:, :, 1],
                            op0=OP.is_equal,
                            op1=OP.mult,
                            accum_out=T92[:, 0:1],
                        )
                        nc.vector.tensor_reduce(
                            T92[:, 1:2], COLR[:, :, 2], axis=AX.X, op=OP.add
                        )
                        # collapse (9,2) -> (1,9,2): ARGr / SUr on partition 0
                        TROW = pool.tile([1, NR, 2], F32)
                        nc.sync.dma_start(out=TROW[0:1, :, :], in_=T92[:, :])
                        ARGr = TROW[0:1, :, 0]
                        SUr8 = TROW[0:1, 0:K, 1]
                    if stage >= 4:
                        # accept_j = eq | (leniency>1 & exp(lgd)*len > pp*S)
                        EQ = pool.tile([1, K], F32)
                        nc.vector.tensor_tensor(
                            EQ[0:1, :],
                            TROW[0:1, 0:K, 0],
                            CON[:, O_DSC : O_DSC + K],
                            op=OP.is_equal,
                        )
                        # lenient: S_j < exp(lgd_j)*len/pp_j (host threshold)
                        LEN = pool.tile([1, K], F32)
                        nc.vector.tensor_tensor(
                            LEN[0:1, :], SUr8, CON[:, O_LGD : O_LGD + K],
                            op=OP.is_lt,
                        )
                        ACC = pool.tile([1, K], F32)
                        nc.vector.tensor_tensor(
                            ACC[0:1, :], EQ[0:1, :], LEN[0:1, :], op=OP.max
                        )
                        TMP = pool.tile([1, K], F32)
                        nc.vector.scalar_tensor_tensor(
                            TMP[0:1, :],
                            ACC[0:1, :],
                            float(K),
                            CON[:, O_POS : O_POS + K],
                            op0=OP.mult,
                            op1=OP.add,
                        )
                        NEWF = pool.tile([1, 16], F32)
                        NM = NEWF[0:1, 9:10]
                        nc.vector.tensor_reduce(NM, TMP[0:1, :], axis=AX.X, op=OP.min)

                        # new ids: pos<nm -> draft_pad; pos==nm -> argmax
                        T1 = pool.tile([1, NR], F32)
                        nc.vector.scalar_tensor_tensor(
                            T1[0:1, :],
                            CON[:, O_POS : O_POS + NR],
                            NM,
                            CON[:, O_DPAD : O_DPAD + NR],
                            op0=OP.is_lt,
                            op1=OP.mult,
                        )
                        T2 = pool.tile([1, NR], F32)
                        nc.vector.scalar_tensor_tensor(
                            T2[0:1, :],
                            CON[:, O_POS : O_POS + NR],
                            NM,
                            ARGr,
                            op0=OP.is_equal,
                            op1=OP.mult,
                        )
                        nc.vector.tensor_tensor(
                            NEWF[0:1, 0:9], T1[0:1, :], T2[0:1, :], op=OP.add
                        )
                        nc.sync.dma_start(out=sout[0:10], in_=NEWF[0:1, 0:10])
                    if stage >= 5:
                        # per-(row,chunk) scale (j<nm)/S_j at width 64, spread
                        # to 64 partitions by DMA, applied on the Scalar engine
                        SINV64 = pool.tile([1, K, CH], F32)
                        nc.vector.reciprocal(
                            SINV64[0:1, :, :],
                            TROW[0:1, 0:K, 1:2].to_broadcast([1, K, CH]),
                        )
                        SCL64r = pool.tile([1, K, CH], F32)
                        nc.vector.scalar_tensor_tensor(
                            SCL64r[0:1, :, :],
                            CON[:, O_POS64 : O_POS64 + K * CH].rearrange(
                                "p (i c) -> p i c", c=CH
                            ),
                            NM,
                            SINV64[0:1, :, :],
                            op0=OP.is_lt,
                            op1=OP.mult,
                        )
                        SCL64 = pool.tile([K * CH, 1], F32)
                        nc.sync.dma_start(out=SCL64[:, :], in_=SCL64r[0:1, :, :])
                        OUTS = pool.tile([K * CH, F], F32)
                        nc.scalar.activation(
                            OUTS[:, :],
                            EXP[0 : K * CH, :],
                            ACT.Copy,
                            scale=SCL64[:, 0:1],
                        )
                        nc.sync.dma_start(
                            out=spec[:, :].rearrange("r (c m) -> (r c) m", c=CH),
                            in_=OUTS[:, :],
                        )

                if "copy" in parts:
                    BT = pool.tile([128, ROWS_PER_CORE * V // 128], F32, tag="copybuf")
                    nc.sync.dma_start(
                        out=BT[:, :],
                        in_=psrc[:, :].rearrange("r (q m) -> (r q) m", q=2),
                    )
                    nc.sync.dma_start(
                        out=pdst[:, :].rearrange("r (q m) -> (r q) m", q=2),
                        in_=BT[:, :],
                    )

    nc.compile()
    return nc


_NC_CACHE = {}


def _get_nc(leniency=2):
    key = int(leniency)
    if key not in _NC_CACHE:
        _NC_CACHE[key] = build_nc(leniency=key)
    return _NC_CACHE[key]


def make_in_maps(target_logits, probs, input_ids, leniency):
    """Build the 8 per-core input dicts from full (unsharded) inputs."""
    tl = np.asarray(target_logits, dtype=np.float32)[0]
    pb = np.asarray(probs, dtype=np.float32)[0]
    ids = np.asarray(input_ids)[0].astype(np.int64)
    draft = ids[R:L]  # (K,)

    sp2 = np.zeros((2 * NP + 1, F), dtype=np.float32)
    sp2[0:NP, :] = tl[R - 1 : L, :].reshape(NP, F)  # logits rows chunked
    rows = np.arange(K)
    con = np.zeros(F, dtype=np.float32)
    con[O_DSC : O_DSC + K] = draft.astype(np.float32)
    lgd = tl[R - 1 + rows, draft].astype(np.float64)
    pp = pb[R - 1 + rows, draft].astype(np.float64)
    if int(leniency) > 1:
        with np.errstate(divide="ignore"):
            thr = np.exp(lgd) * float(leniency) / pp
    else:
        thr = np.zeros(K)
    con[O_LGD : O_LGD + K] = thr.astype(np.float32)
    con[O_POS : O_POS + NR] = np.arange(NR, dtype=np.float32)
    con[O_DPAD : O_DPAD + K] = draft.astype(np.float32)  # last entry stays 0
    con[O_POS64 : O_POS64 + K * CH] = np.repeat(
        np.arange(K, dtype=np.float32), CH
    )
    sp2[NP, :] = con
    sp2[NP + 1 :, :] = np.tile(
        np.arange(V, dtype=np.float32).reshape(CH, F), (NR, 1)
    )

    in_maps = []
    for c in range(NCORES):
        in_maps.append(
            {
                "psrc": np.ascontiguousarray(
                    pb[c * ROWS_PER_CORE : (c + 1) * ROWS_PER_CORE, :]
                ),
                "sp2": sp2,
            }
        )
    return in_maps


def assemble(results, input_ids):
    """Combine per-core outputs into the full (id_res, prob_res, n_match)."""
    ids = np.asarray(input_ids)
    prob_res = np.empty((1, L, V), dtype=np.float32)
    for c in range(NCORES):
        prob_res[0, c * ROWS_PER_CORE : (c + 1) * ROWS_PER_CORE, :] = results[c]["pdst"]
    prob_res[0, R:L, :] = results[0]["spec"]
    sout = results[0]["sout"]
    new_ids = np.rint(sout[0:9]).astype(ids.dtype)
    id_res = np.concatenate([ids[0, :R], new_ids])[None, :]
    n_match = np.rint(sout[9]).astype(np.int32).reshape(())
    return id_res, prob_res, n_match


def kernel(target_logits, probs, input_ids, review_index, leniency):
    assert int(review_index) == R, f"kernel hardcodes review_index={R}"
    from concourse.bass_utils import run_bass_kernel_spmd

    nc = _get_nc(int(leniency))
    in_maps = make_in_maps(target_logits, probs, input_ids, leniency)
    res = run_bass_kernel_spmd(nc, in_maps, core_ids=list(range(NCORES)))
    return assemble(res.results, input_ids)


# revision 19
# speedup vs baseline: 1.0720x; 1.0720x over previous
"""Trainium2 Bass kernel for CSDrafting speculative-decoding verification.

Problem: B=1, L=520, V=32128, review_index r=512, K=8 draft tokens, 8 cores.

Work decomposition (row-sharding instead of the vocab-sharding hint — no
collectives needed):
  - prob_res rows 0..511 are a verbatim copy of probs rows 0..511: sharded
    row-wise, 64 rows per core, as 8 independent 1 MB HBM->SBUF->HBM DMA
    pairs (they overlap; only the 2-deep chain latency shows).
  - The "special" math (softmax+argmax of logits rows 511..519, accept flags,
    n_match, masked softmax rows 512..519, new ids) runs redundantly on every
    core; the host takes core 0's copy.

This axon terminal charges a large fixed latency (~30-60us) per instruction
(PE matmul ~200us, broadcast-operand reads +45us) and cannot load custom
DVE-table ops, so the design minimizes the dependency-chain depth using only
standard BIR ops:
  - logits rows live as (9 rows x 8 chunks = 72 partitions, 4016 free) so
    every per-row comparison uses per-partition scalars (no broadcasts),
  - exp + per-chunk sumexp fuse into one activation with accum_out,
  - (mask==max)*index + its reduction fuse into one scalar_tensor_tensor
    with accum_out; same for the cross-chunk winner select,
  - cross-partition steps use two tiny SBUF->SBUF DMA partition-collapses,
  - the lenient test compares the device row-sum S_j against a host-packed
    threshold exp(lgd_j)*leniency/pp_j (saves the Ln activation),
  - n_match = min_j(j + K*accept_j) replaces the cumprod scan,
  - the per-row output scale expands to 64 partitions via one free-dim
    broadcast op + one partition-spread DMA, and the final masked-softmax
    multiply runs on the Scalar engine (activation Copy with per-partition
    scale), off the Vector engine's critical chain.
  - the 16 per-draft scalars the reference gathers (probs/logits at draft
    ids) are host-gathered into the constants row (pure input marshaling;
    all O(V) compute stays on device).
"""

import sys

sys.path.insert(0, "/opt/trn_rl_repo")

import numpy as np

import concourse.bacc as bacc
import concourse.mybir as mybir
from concourse.tile import TileContext

F32 = mybir.dt.float32
AX = mybir.AxisListType
OP = mybir.AluOpType
ACT = mybir.ActivationFunctionType

V = 32128
L = 520
R = 512
K = 8
NR = K + 1  # 9 rows under review
CH = 8  # vocab chunks per row
F = V // CH  # 4016 free elems per chunk
NP = NR * CH  # 72 partitions of logits data
NCORES = 8
ROWS_PER_CORE = R // NCORES  # 64

# offsets inside the constants row (row 72 of sp2)
O_DSC = 0  # [0:8]    draft ids (raw)
O_LGD = 8  # [8:16]   exp(lgd)*leniency/pp  (lenient accept threshold)
O_POS = 24  # [24:33]  arange(9)
O_DPAD = 33  # [33:42]  draft ids raw, last = 0
O_POS64 = 64  # [64:128] q//8 for q in 0..63


def build_nc(niter=1, internal_bulk=False, parts=("copy", "special"), leniency=2, stage=5):
    nc = bacc.Bacc("TRN2", target_bir_lowering=False, debug=False, num_devices=NCORES)

    if internal_bulk:
        psrc = nc.dram_tensor("ipsrc", [ROWS_PER_CORE, V], F32)
        pdst = nc.dram_tensor("ipdst", [ROWS_PER_CORE, V], F32)
        spec = nc.dram_tensor("ispec", [K, V], F32)
    else:
        psrc = nc.dram_tensor("psrc", [ROWS_PER_CORE, V], F32, kind="ExternalInput")
        pdst = nc.dram_tensor("pdst", [ROWS_PER_CORE, V], F32, kind="ExternalOutput")
        spec = nc.dram_tensor("spec", [K, V], F32, kind="ExternalOutput")
    sp2 = nc.dram_tensor("sp2", [NP * (F + 128) + NP * F], F32, kind="ExternalInput")
    sout = nc.dram_tensor("sout", [16], F32, kind="ExternalOutput")

    lgate = 1.0 if leniency > 1 else 0.0
    lenf = float(leniency)

    with TileContext(nc) as tc:
        with (
            tc.tile_pool(name="pool", bufs=2) as pool,
            tc.tile_pool(name="bulk", bufs=1) as bulkp,
        ):
            for _ in range(niter):
                if "special" in parts:
                    # logits chunks with 128 extras columns; the constants
                    # live in partition 0's extras (engine-read-legal), so one
                    # DMA loads logits + constants together
                    SPT = pool.tile([NP, F + 128], F32)
                    nc.sync.dma_start(
                        out=SPT[:, :],
                        in_=sp2[0 : NP * (F + 128)].rearrange("(p m) -> p m", p=NP),
                    )
                    IOT = pool.tile([NP, F], F32)
                    nc.sync.dma_start(
                        out=IOT[:, :],
                        in_=sp2[NP * (F + 128) :].rearrange("(p m) -> p m", p=NP),
                    )
                    CON = SPT[0:1, F : F + 128]

                    COLT = pool.tile([NP, 3], F32)
                    # exp + per-chunk sumexp in one op
                    EXP = pool.tile([NP, F], F32)
                    nc.scalar.activation(
                        EXP[:, :], SPT[:, 0:F], ACT.Exp, accum_out=COLT[:, 2:3]
                    )
                    if stage >= 2:
                        # per-chunk max, then fused (x==max)*index with accum
                        nc.vector.tensor_reduce(
                            COLT[0:NP, 0:1], SPT[0:NP, 0:F], axis=AX.X, op=OP.max
                        )
                        nc.vector.scalar_tensor_tensor(
                            IOT[:, :],
                            SPT[0:NP, 0:F],
                            COLT[0:NP, 0:1],
                            IOT[:, :],
                            op0=OP.is_equal,
                            op1=OP.mult,
                            accum_out=COLT[0:NP, 1:2],
                        )
                    if stage >= 3:
                        # collapse (72,3) -> (9 rows, 8 chunks, 3)
                        COLR = pool.tile([NR, CH, 3], F32)
                        nc.sync.dma_start(out=COLR[:, :, :], in_=COLT[0:NP, :])
                        G9 = pool.tile([NR, 1], F32)
                        nc.vector.tensor_reduce(
                            G9[:, :], COLR[:, :, 0], axis=AX.X, op=OP.max
                        )
                        T92 = pool.tile([NR, 2], F32)
                        W9 = pool.tile([NR, CH], F32)
                        nc.vector.scalar_tensor_tensor(
                            W9[:, :],
                            COLR[:, :, 0],
                            G9[:, 0:1],
                            COLR[:, :, 1],
                            op0=OP.is_equal,
                            op1=OP.mult,
                            accum_out=T92[:, 0:1],
                        )
                        nc.vector.tensor_reduce(
                            T92[:, 1:2], COLR[:, :, 2], axis=AX.X, op=OP.add
                        )
                        # collapse (9,2) -> (1,9,2): ARGr / SUr on partition 0
                        TROW = pool.tile([1, NR, 2], F32)
                        nc.sync.dma_start(out=TROW[0:1, :, :], in_=T92[:, :])
                        ARGr = TROW[0:1, :, 0]
                        SUr8 = TROW[0:1, 0:K, 1]
                    if stage >= 4:
                        # accept_j = eq | (leniency>1 & exp(lgd)*len > pp*S)
                        EQ = pool.tile([1, K], F32)
                        nc.vector.tensor_tensor(
                            EQ[0:1, :],
                            TROW[0:1, 0:K, 0],
                            CON[:, O_DSC : O_DSC + K],
                            op=OP.is_equal,
                        )
                        # lenient: S_j < exp(lgd_j)*len/pp_j (host threshold)
                        LEN = pool.tile([1, K], F32)
                        nc.vector.tensor_tensor(
                            LEN[0:1, :], SUr8, CON[:, O_LGD : O_LGD + K],
                            op=OP.is_lt,
                        )
                        ACC = pool.tile([1, K], F32)
                        nc.vector.tensor_tensor(
                            ACC[0:1, :], EQ[0:1, :], LEN[0:1, :], op=OP.max
                        )
                        TMP = pool.tile([1, K], F32)
                        nc.vector.scalar_tensor_tensor(
                            TMP[0:1, :],
                            ACC[0:1, :],
                            float(K),
                            CON[:, O_POS : O_POS + K],
                            op0=OP.mult,
                            op1=OP.add,
                        )
                        NEWF = pool.tile([1, 16], F32)
                        NM = NEWF[0:1, 9:10]
                        nc.vector.tensor_reduce(NM, TMP[0:1, :], axis=AX.X, op=OP.min)

                        # new ids: pos<nm -> draft_pad; pos==nm -> argmax
                        T1 = pool.tile([1, NR], F32)
                        nc.vector.scalar_tensor_tensor(
                            T1[0:1, :],
                            CON[:, O_POS : O_POS + NR],
                            NM,
                            CON[:, O_DPAD : O_DPAD + NR],
                            op0=OP.is_lt,
                            op1=OP.mult,
                        )
                        T2 = pool.tile([1, NR], F32)
                        nc.vector.scalar_tensor_tensor(
                            T2[0:1, :],
                            CON[:, O_POS : O_POS + NR],
                            NM,
                            ARGr,
                            op0=OP.is_equal,
                            op1=OP.mult,
                        )
                        nc.vector.tensor_tensor(
                            NEWF[0:1, 0:9], T1[0:1, :], T2[0:1, :], op=OP.add
                        )
                        nc.sync.dma_start(out=sout[0:10], in_=NEWF[0:1, 0:10])
                    if stage >= 5:
                        # per-(row,chunk) scale (j<nm)/S_j at width 64, spread
                        # to 64 partitions by DMA, applied on the Scalar engine
                        SINV64 = pool.tile([1, K, CH], F32)
                        nc.vector.reciprocal(
                            SINV64[0:1, :, :],
                            TROW[0:1, 0:K, 1:2].to_broadcast([1, K, CH]),
                        )
                        SCL64r = pool.tile([1, K, CH], F32)
                        nc.vector.scalar_tensor_tensor(
                            SCL64r[0:1, :, :],
                            CON[:, O_POS64 : O_POS64 + K * CH].rearrange(
                                "p (i c) -> p i c", c=CH
                            ),
                            NM,
                            SINV64[0:1, :, :],
                            op0=OP.is_lt,
                            op1=OP.mult,
                        )
                        SCL64 = pool.tile([K * CH, 1], F32)
                        nc.sync.dma_start(out=SCL64[:, :], in_=SCL64r[0:1, :, :])
                        OUTS = pool.tile([K * CH, F], F32)
                        nc.scalar.activation(
                            OUTS[:, :],
                            EXP[0 : K * CH, :],
                            ACT.Copy,
                            scale=SCL64[:, 0:1],
                        )
                        nc.sync.dma_start(
                            out=spec[:, :].rearrange("r (c m) -> (r c) m", c=CH),
                            in_=OUTS[:, :],
                        )

                if "copy" in parts:
                    BT = bulkp.tile([128, ROWS_PER_CORE * V // 128], F32, tag="copybuf")
                    nc.sync.dma_start(
                        out=BT[:, :],
                        in_=psrc[:, :].rearrange("r (q m) -> (r q) m", q=2),
                    )
                    nc.sync.dma_start(
                        out=pdst[:, :].rearrange("r (q m) -> (r q) m", q=2),
                        in_=BT[:, :],
                    )

    nc.compile()
    return nc


_NC_CACHE = {}


def _get_nc(leniency=2):
    key = int(leniency)
    if key not in _NC_CACHE:
        _NC_CACHE[key] = build_nc(leniency=key)
    return _NC_CACHE[key]


def make_in_maps(target_logits, probs, input_ids, leniency):
    """Build the 8 per-core input dicts from full (unsharded) inputs."""
    tl = np.asarray(target_logits, dtype=np.float32)[0]
    pb = np.asarray(probs, dtype=np.float32)[0]
    ids = np.asarray(input_ids)[0].astype(np.int64)
    draft = ids[R:L]  # (K,)

    blockA = np.zeros((NP, F + 128), dtype=np.float32)
    blockA[:, 0:F] = tl[R - 1 : L, :].reshape(NP, F)  # logits rows chunked
    rows = np.arange(K)
    con = np.zeros(128, dtype=np.float32)
    con[O_DSC : O_DSC + K] = draft.astype(np.float32)
    lgd = tl[R - 1 + rows, draft].astype(np.float64)
    pp = pb[R - 1 + rows, draft].astype(np.float64)
    if int(leniency) > 1:
        with np.errstate(divide="ignore"):
            thr = np.exp(lgd) * float(leniency) / pp
    else:
        thr = np.zeros(K)
    con[O_LGD : O_LGD + K] = thr.astype(np.float32)
    con[O_POS : O_POS + NR] = np.arange(NR, dtype=np.float32)
    con[O_DPAD : O_DPAD + K] = draft.astype(np.float32)  # last entry stays 0
    con[O_POS64 : O_POS64 + K * CH] = np.repeat(
        np.arange(K, dtype=np.float32), CH
    )
    blockA[0, F : F + 128] = con
    blockB = np.tile(np.arange(V, dtype=np.float32).reshape(CH, F), (NR, 1))
    sp2 = np.concatenate([blockA.ravel(), blockB.ravel()])

    in_maps = []
    for c in range(NCORES):
        in_maps.append(
            {
                "psrc": np.ascontiguousarray(
                    pb[c * ROWS_PER_CORE : (c + 1) * ROWS_PER_CORE, :]
                ),
                "sp2": sp2,
            }
        )
    return in_maps


def assemble(results, input_ids):
    """Combine per-core outputs into the full (id_res, prob_res, n_match)."""
    ids = np.asarray(input_ids)
    prob_res = np.empty((1, L, V), dtype=np.float32)
    for c in range(NCORES):
        prob_res[0, c * ROWS_PER_CORE : (c + 1) * ROWS_PER_CORE, :] = results[c]["pdst"]
    prob_res[0, R:L, :] = results[0]["spec"]
    sout = results[0]["sout"]
    new_ids = np.rint(sout[0:9]).astype(ids.dtype)
    id_res = np.concatenate([ids[0, :R], new_ids])[None, :]
    n_match = np.rint(sout[9]).astype(np.int32).reshape(())
    return id_res, prob_res, n_match


def kernel(target_logits, probs, input_ids, review_index, leniency):
    assert int(review_index) == R, f"kernel hardcodes review_index={R}"
    from concourse.bass_utils import run_bass_kernel_spmd

    nc = _get_nc(int(leniency))
    in_maps = make_in_maps(target_logits, probs, input_ids, leniency)
    res = run_bass_kernel_spmd(nc, in_maps, core_ids=list(range(NCORES)))
    return assemble(res.results, input_ids)
